# revision 1
# baseline (speedup 1.0000x reference)
"""Trainium2 Bass kernel for nn_Block_58497454571919 (dense transformer block).

Reference semantics (B=4, S=2048, D=2048, H=16, Dh=128, DFF=8192):
  X = x @ W1.T + b1 ; Q,K,V = split(X)
  per (b,h): scores[d,e] = sum_s Q[b,s,hd]K[b,s,he] / sqrt(S)  (feature-attention)
             w = softmax(scores, axis=e);  out[d,s] = sum_e w[d,e] V[b,s,he]
  attn_pre[b, h*128+d, s] = out[d,s]   (raw memory reshape)
  a = attn_pre @ W2.T + b2 ; t1 = a + x ; x1 = global_scalar_LN(t1, lnw1, lnb1)
  m = gelu_tanh(x1 @ fc.T + fcb) @ proj.T + projb ; t2 = m + x1
  y = global_scalar_LN(t2, lnw2, lnb2)

Distribution over 8 cores: core c owns heads {2c, 2c+1} == output rows
[256c, 256c+256) of every batch. The QKV projection for those heads needs all
tokens (full x); W2/LN/FFN are row-parallel on the core's 4*256=1024 rows.
The only cross-core data dependency is the global-scalar LayerNorm mean/var:
two tiny AllReduces of (sum, sumsq).

LN1 is algebraically deferred so its AllReduce overlaps the FFN1 matmuls:
ln1_w is folded into fc on the host, FFN1 contracts the *unnormalized*
residual t1, and the normalization enters through the gelu activation's
per-partition scale (rstd) and bias (kbf - mu*rstd*kw, with kw/kbf host
matvecs of fc against ln1_w/ln1_b).

On-device layouts (all "transposed" so no device transposes are needed):
  QK[b]   [128 s_in, 16 s_out, 512 (q 256|k 256)] bf16
  VT[b]   [128 vf_in, 2 head, 2048 s]             bf16
  attnT   [128 s_in, 16 s_out, 1024 i]            bf16   i = b*256 + hl*128 + d
  t1b/x1' [128 n_in, 16 n_out, 1024 i]            bf16   (x1' = x1 + proj_b)
"""
import math
import os
import sys
import types

import numpy as np
import ml_dtypes

import concourse.bass as bass
import concourse.bacc as bacc
import concourse.mybir as mybir
import concourse.tile as tile
from concourse import bass_utils
from concourse.masks import make_identity

F32 = mybir.dt.float32
BF16 = mybir.dt.bfloat16
AF = mybir.ActivationFunctionType
OP = mybir.AluOpType

N_CORES = 8
B, S, D, H, DH, DFF = 4, 2048, 2048, 16, 128, 8192
P = 128
EPS = 1e-12
SM_SCALE = 1.0 / math.sqrt(S)
N_GLOB = float(B * S * D)          # 16777216 elements in each layernorm
N_PGROUPS = 1024.0                 # 8 cores * 128 partitions

TRACE = False          # set by test.py to capture an NTFF profile
LAST_RESULT = None     # BassKernelResults stash for test.py


def _register_ntff_hook():
    """The agent image's antenv lacks axon_hooks; inject it so trace=True works."""
    if "antenv.axon_hooks" in sys.modules:
        return
    mod = types.ModuleType("antenv.axon_hooks")
    mod._hook = None
    mod.set_axon_ntff_profile_hook = lambda h: setattr(mod, "_hook", h)
    mod.get_axon_ntff_profile_hook = lambda: mod._hook
    sys.modules["antenv.axon_hooks"] = mod
    import antenv

    antenv.axon_hooks = mod
    try:
        from trn_agent_boot.trn_boot import _ntff_profile_via_ctypes

        mod.set_axon_ntff_profile_hook(
            _ntff_profile_via_ctypes("/opt/axon/libaxon_pjrt.so")
        )
    except Exception:
        pass


def build_program():
    nc = bacc.Bacc("TRN2", target_bir_lowering=False, debug=False, num_devices=N_CORES)

    def din(name, shape, dtype):
        return nc.dram_tensor(name, shape, dtype, kind="ExternalInput").ap()

    ins = {
        "xq": din("xq", [B, 4, P, 16, 512], BF16),     # x^T tiles [b, sb, d_in, d_out, s]
        "w1qk": din("w1qk", [P, 16, 512], BF16),       # [d_in, d_out, (q|k) feat]
        "b1qk": din("b1qk", [P, 512], F32),            # replicated over partitions
        "w1v": din("w1v", [P, 16, 256], BF16),         # [d_in, d_out, vfeat]
        "b1v": din("b1v", [P, 2], F32),                # [vf_in, head]
        "w2": din("w2", [16, P, 16, 128], BF16),       # [n_blk, s_in, s_out, n]
        "xres": din("xres", [P, 16, 1024], F32),       # (x + b2)^T slice [n_in, n_out, i]
        "fc": din("fc", [64, P, 16, 128], BF16),       # ln1_w-scaled fc^T tiles
        "kw": din("kw", [P, 64], F32),                 # fc @ ln1_w     [f_in, f_blk]
        "kbf": din("kbf", [P, 64], F32),               # fc @ ln1_b + fc_b
        "proj": din("proj", [16, P, 64, 128], BF16),   # [n_blk, f_in, f_out, n]
        "projb": din("projb", [P, 16], F32),           # [n_in, n_out]
        "lnw1": din("lnw1", [P, 16], F32),
        "lnb1": din("lnb1", [P, 16], F32),
        "lnw2": din("lnw2", [P, 16], F32),
        "lnb2": din("lnb2", [P, 16], F32),
    }
    y_out = nc.dram_tensor("y", [P, 16, 1024], F32, kind="ExternalOutput").ap()

    with tile.TileContext(nc) as tc:
        _emit(nc, tc, ins, y_out)
    nc.compile()
    return nc


def _emit(nc, tc, I, y_out):
    with (
        tc.tile_pool(name="consts", bufs=1) as consts,
        tc.tile_pool(name="stats", bufs=1) as stats,
        tc.tile_pool(name="small", bufs=3) as small,
        tc.tile_pool(name="dram", bufs=1, space="DRAM") as dram,
        tc.tile_pool(name="ps_red", bufs=1, space="PSUM") as ps_red,
        tc.tile_pool(name="t1pool", bufs=1) as t1_pool,
    ):
        # t1b first so its pool exists before anything else writes it
        t1b = t1_pool.tile([P, 16, 1024], BF16, name="t1b")
        stats1 = stats.tile([P, 32, 6], F32, name="stats1")
        stats2 = stats.tile([P, 32, 6], F32, name="stats2")
        x1_dram = dram.tile([P, 16, 1024], BF16, name="x1_dram")
        t2_dram = dram.tile([P, 16, 1024], BF16, name="t2_dram")

        with (
            tc.tile_pool(name="attn", bufs=1) as attn_pool,
            tc.tile_pool(name="ps_big", bufs=4, space="PSUM") as ps_big,
            tc.tile_pool(name="ps_sm", bufs=3, space="PSUM") as ps_sm,
        ):
            attnT = attn_pool.tile([P, 16, 1024], BF16, name="attnT")

            with (
                tc.tile_pool(name="w1pool", bufs=1) as w1_pool,
                tc.tile_pool(name="xq", bufs=2) as xq_pool,
                tc.tile_pool(name="qkpool", bufs=2) as qk_pool,
            ):
                # critical-path DMAs first: QKV weights + first x tiles
                w1qk_sb = w1_pool.tile([P, 16, 512], BF16, name="w1qk_sb")
                nc.sync.dma_start(w1qk_sb[:, 0:8, :], I["w1qk"][:, 0:8, :])
                nc.sync.dma_start(w1qk_sb[:, 8:16, :], I["w1qk"][:, 8:16, :])
                w1v_sb = w1_pool.tile([P, 16, 256], BF16, name="w1v_sb")
                nc.sync.dma_start(w1v_sb[:], I["w1v"][:])
                b1qk_sb = w1_pool.tile([P, 512], F32, name="b1qk_sb")
                nc.sync.dma_start(b1qk_sb[:], I["b1qk"][:])
                b1v_sb = w1_pool.tile([P, 2], F32, name="b1v_sb")
                nc.sync.dma_start(b1v_sb[:], I["b1v"][:])
                ident = consts.tile([P, P], BF16, name="ident")
                make_identity(nc, ident[:])
                ones = consts.tile([P, 1], F32, name="ones")
                nc.vector.memset(ones[:], 1.0)
                epsb = consts.tile([P, 1], F32, name="epsb")
                nc.vector.memset(epsb[:], EPS)

                def load_consts():
                    tiles = {}
                    for nm in ("kw", "kbf", "projb", "lnw1", "lnb1", "lnw2", "lnb2"):
                        t = consts.tile(list(I[nm].shape), F32, name=f"{nm}_sb")
                        nc.sync.dma_start(t[:], I[nm][:])
                        tiles[nm] = t
                    return tiles

                def phase12(b):
                    QK = qk_pool.tile([P, 16, 512], BF16, name="QK", tag="QK")
                    VT = qk_pool.tile([P, 2, S], BF16, name="VT", tag="VT")
                    for sb in range(4):
                        xt = xq_pool.tile([P, 16, 512], BF16, name="xt", tag="xt")
                        eng = nc.gpsimd if (b == 0 and sb == 0) else nc.sync
                        eng.dma_start(xt[:, 0:8, :], I["xq"][b, sb, :, 0:8, :])
                        nc.sync.dma_start(xt[:, 8:16, :], I["xq"][b, sb, :, 8:16, :])
                        for ss in range(4):
                            pqk = ps_big.tile([P, 512], F32, name="pqk", tag="psbig")
                            for do in range(16):
                                nc.tensor.matmul(
                                    pqk[:], xt[:, do, ss * 128:(ss + 1) * 128],
                                    w1qk_sb[:, do, :], start=(do == 0), stop=(do == 15),
                                )
                            nc.vector.tensor_tensor(
                                QK[:, sb * 4 + ss, :], pqk[:], b1qk_sb[:], OP.add)
                        for vo in range(2):
                            pv = ps_big.tile([P, 512], F32, name="pv", tag="psbig")
                            for do in range(16):
                                nc.tensor.matmul(
                                    pv[:], w1v_sb[:, do, vo * 128:(vo + 1) * 128],
                                    xt[:, do, :], start=(do == 0), stop=(do == 15),
                                )
                            nc.vector.tensor_scalar(
                                VT[:, vo, sb * 512:(sb + 1) * 512], pv[:],
                                b1v_sb[:, vo:vo + 1], None, OP.add)
                    for hl in range(2):
                        pscore = ps_sm.tile([P, P], F32, name="pscore", tag="pssm")
                        for so in range(16):
                            nc.tensor.matmul(
                                pscore[:], QK[:, so, hl * 128:(hl + 1) * 128],
                                QK[:, so, 256 + hl * 128:256 + (hl + 1) * 128],
                                start=(so == 0), stop=(so == 15),
                            )
                        # 1/sqrt(S) is folded into the Q weights on the host,
                        # so psum scores are pre-scaled: exp(x - max) directly.
                        negmax = small.tile([P, 1], F32, name="negmax", tag="negmax")
                        nc.vector.reduce_max(negmax[:], pscore[:],
                                             axis=mybir.AxisListType.X, negate=True)
                        wexp = small.tile([P, P], F32, name="wexp", tag="wexp")
                        rowsum = small.tile([P, 1], F32, name="rowsum", tag="rowsum")
                        nc.scalar.activation(wexp[:], pscore[:], AF.Exp,
                                             bias=negmax[:], scale=1.0,
                                             accum_out=rowsum[:])
                        rinv = small.tile([P, 1], F32, name="rinv", tag="rinv")
                        nc.vector.reciprocal(rinv[:], rowsum[:])
                        wnorm = small.tile([P, P], BF16, name="wnorm", tag="wnorm")
                        nc.vector.tensor_scalar_mul(wnorm[:], wexp[:], rinv[:])
                        pwt = ps_sm.tile([P, P], BF16, name="pwt", tag="pssm")
                        nc.tensor.transpose(pwt[:], wnorm[:], ident[:])
                        wT = small.tile([P, P], BF16, name="wT", tag="wT")
                        nc.vector.tensor_copy(wT[:], pwt[:])
                        for so in range(16):
                            pat = ps_sm.tile([P, P], F32, name="pat", tag="pssm")
                            nc.tensor.matmul(
                                pat[:], VT[:, hl, so * 128:(so + 1) * 128], wT[:],
                                start=True, stop=True,
                            )
                            nc.vector.tensor_copy(
                                attnT[:, so, b * 256 + hl * 128:b * 256 + (hl + 1) * 128],
                                pat[:])

                def w2_half(bp, w2_pool):
                    for nb in range(16):
                        w2t = w2_pool.tile([P, 16, 128], BF16, name="w2t", tag="w2t")
                        nc.sync.dma_start(w2t[:], I["w2"][nb])
                        pw2 = ps_big.tile([P, 512], F32, name="pw2", tag="psbig")
                        for so in range(16):
                            nc.tensor.matmul(
                                pw2[:], w2t[:, so, :],
                                attnT[:, so, bp * 512:(bp + 1) * 512],
                                start=(so == 0), stop=(so == 15),
                            )
                        xr = w2_pool.tile([P, 512], F32, name="xr", tag="xr")
                        nc.sync.dma_start(xr[:], I["xres"][:, nb, bp * 512:(bp + 1) * 512])
                        t1s = t1b[:, nb, bp * 512:(bp + 1) * 512]
                        nc.vector.tensor_tensor(t1s, pw2[:], xr[:], OP.add)
                        nc.vector.bn_stats(stats1[:, nb * 2 + bp, :], t1s)

                with tc.tile_pool(name="w2pool", bufs=2) as w2_pool:
                    phase12(0)
                    C = load_consts()
                    phase12(1)
                    w2_half(0, w2_pool)
                    phase12(2)
                    phase12(3)
                    dum1 = stats.tile([P, 1], F32, name="dum1")
                    nc.scalar.activation(dum1[:], epsb[:], AF.Sqrt)
                    w2_half(1, w2_pool)

        # ---- LN1 scalars (AllReduce overlaps the FFN1 matmuls below) ----
        mu1, rstd1 = _ln_allreduce(nc, stats, dram, ps_red, ones, epsb, stats1, "ln1")
        murstd1 = stats.tile([P, 1], F32, name="murstd1")
        nc.vector.tensor_tensor(murstd1[:], mu1[:], rstd1[:], OP.mult)
        # gelu bias: kbf - mu*rstd*kw   [128, 64]
        gbias = stats.tile([P, 64], F32, name="gbias")
        nc.vector.tensor_scalar_mul(gbias[:], C["kw"][:], murstd1[:])
        nc.vector.tensor_sub(gbias[:], C["kbf"][:], gbias[:])
        # ============ FFN ============
        with (
            tc.tile_pool(name="hpool", bufs=1) as h_pool,
            tc.tile_pool(name="fcpool", bufs=2) as fc_pool,
            tc.tile_pool(name="projpool", bufs=2) as proj_pool,
            tc.tile_pool(name="hpre", bufs=12) as hpre_pool,
                tc.tile_pool(name="xspool", bufs=3) as xs_pool,
            tc.tile_pool(name="ps_h", bufs=5, space="PSUM") as ps_h,
            tc.tile_pool(name="ps_m", bufs=2, space="PSUM") as ps_m,
        ):
            for ch in range(2):
                hT = h_pool.tile([P, 64, 512], BF16, name="hT", tag="hT")
                for fb in range(64):
                    fct = fc_pool.tile([P, 16, 128], BF16, name="fct", tag="fct")
                    nc.sync.dma_start(fct[:], I["fc"][fb])
                    ph = ps_h.tile([P, 512], F32, name="ph", tag="psh")
                    for do in range(16):
                        nc.tensor.matmul(
                            ph[:], fct[:, do, :],
                            t1b[:, do, ch * 512:(ch + 1) * 512],
                            start=(do == 0), stop=(do == 15),
                        )
                    if ch == 0 and fb < 12:
                        # drain psum via DVE so the PE never stalls while
                        # the LN1 AllReduce is still in flight
                        hp = hpre_pool.tile([P, 512], F32, name="hp", tag="hp")
                        nc.vector.tensor_copy(hp[:], ph[:])
                        gsrc = hp
                    else:
                        gsrc = ph
                    nc.scalar.activation(hT[:, fb, :], gsrc[:], AF.Gelu_apprx_tanh,
                                         bias=gbias[:, fb:fb + 1], scale=rstd1[:])
                if ch == 0:
                    # x1' = rstd*lnw1*t1b + (lnb1 - mu*rstd*lnw1 + projb) -> DRAM
                    # (emitted after FFN1 so these AllReduce-gated DVE ops do
                    # not head-of-line block the psum-drain copies above)
                    s1 = stats.tile([P, 16], F32, name="s1")
                    nc.vector.tensor_scalar_mul(s1[:], C["lnw1"][:], rstd1[:])
                    c1 = stats.tile([P, 16], F32, name="c1")
                    nc.vector.tensor_scalar_mul(c1[:], s1[:], mu1[:])
                    nc.vector.tensor_sub(c1[:], C["lnb1"][:], c1[:])
                    nc.vector.tensor_add(c1[:], c1[:], C["projb"][:])
                    for nb in range(16):
                        xs_t = xs_pool.tile([P, 1024], BF16, name="xs_t", tag="xs_t")
                        nc.vector.tensor_scalar(
                            xs_t[:], t1b[:, nb, :],
                            s1[:, nb:nb + 1], c1[:, nb:nb + 1], OP.mult, OP.add)
                        nc.gpsimd.dma_start(x1_dram[:, nb, :], xs_t[:])
                for nb in range(16):
                    pjt = proj_pool.tile([P, 64, 128], BF16, name="pjt", tag="pjt")
                    nc.sync.dma_start(pjt[:], I["proj"][nb])
                    pm = ps_m.tile([P, 512], F32, name="pm", tag="psm")
                    for fo in range(64):
                        nc.tensor.matmul(
                            pm[:], pjt[:, fo, :], hT[:, fo, :],
                            start=(fo == 0), stop=(fo == 63),
                        )
                    x1r = proj_pool.tile([P, 512], BF16, name="x1r", tag="x1r")
                    nc.gpsimd.dma_start(x1r[:], x1_dram[:, nb, ch * 512:(ch + 1) * 512])
                    t2s = proj_pool.tile([P, 512], BF16, name="t2s", tag="t2s")
                    nc.vector.tensor_tensor(t2s[:], pm[:], x1r[:], OP.add)
                    nc.vector.bn_stats(stats2[:, nb * 2 + ch, :], t2s[:])
                    nc.gpsimd.dma_start(t2_dram[:, nb, ch * 512:(ch + 1) * 512], t2s[:])

        # ============ LN2 (AllReduce) -> output ============
        dum2 = stats.tile([P, 1], F32, name="dum2")
        nc.scalar.activation(dum2[:], epsb[:], AF.Sqrt)
        with tc.tile_pool(name="outpool", bufs=1) as out_pool:
            t2r = out_pool.tile([P, 16, 1024], BF16, name="t2r")
            nc.sync.dma_start(t2r[:], t2_dram[:])
            mu2, rstd2 = _ln_allreduce(nc, stats, dram, ps_red, ones, epsb, stats2, "ln2", cc_in_sync=True)
            s2 = stats.tile([P, 16], F32, name="s2")
            nc.vector.tensor_scalar_mul(s2[:], C["lnw2"][:], rstd2[:])
            c2 = stats.tile([P, 16], F32, name="c2")
            nc.vector.tensor_scalar_mul(c2[:], s2[:], mu2[:])
            nc.vector.tensor_sub(c2[:], C["lnb2"][:], c2[:])
            ys = out_pool.tile([P, 16, 1024], F32, name="ys")
            for g in range(4):
                for j in range(4):
                    nb = 4 * g + j
                    if j % 2 == 0:
                        nc.vector.tensor_scalar(
                            ys[:, nb, :], t2r[:, nb, :],
                            s2[:, nb:nb + 1], c2[:, nb:nb + 1], OP.mult, OP.add)
                    else:
                        nc.scalar.activation(
                            ys[:, nb, :], t2r[:, nb, :], AF.Identity,
                            bias=c2[:, nb:nb + 1], scale=s2[:, nb:nb + 1])
                nc.sync.dma_start(y_out[:, 4 * g:4 * g + 4, :], ys[:, 4 * g:4 * g + 4, :])


def _ln_allreduce(nc, stats, dram, ps_red, ones, epsb, stats_t, tag, cc_in_sync=False):
    """bn_stats tiles -> global scalar mean + rstd via cross-core AllReduce."""
    mv = stats.tile([P, 2], F32, name=f"mv_{tag}")
    nc.vector.bn_aggr(mv[:], stats_t[:])
    # red_in[:,0] = mean_p ; red_in[:,1] = meansq_p = var_p + mean_p^2
    red_in = stats.tile([P, 2], F32, name=f"red_in_{tag}")
    nc.vector.tensor_copy(red_in[:, 0:1], mv[:, 0:1])
    nc.vector.tensor_tensor(red_in[:, 1:2], mv[:, 0:1], mv[:, 0:1], OP.mult)
    nc.vector.tensor_tensor(red_in[:, 1:2], red_in[:, 1:2], mv[:, 1:2], OP.add)
    pred = ps_red.tile([1, 2], F32, name=f"pred_{tag}", tag="psred")
    nc.tensor.matmul(pred[:], ones[:], red_in[:], start=True, stop=True)
    cc_sb = stats.tile([1, 8], F32, name=f"cc_sb_{tag}")
    nc.vector.memset(cc_sb[:], 0.0)
    nc.vector.tensor_copy(cc_sb[:, 0:2], pred[:])
    cc_in = dram.tile([1, 8], F32, name=f"cc_in_{tag}")
    cc_out = dram.tile([1, 8], F32, name=f"cc_out_{tag}", addr_space="Shared")
    (nc.sync if cc_in_sync else nc.gpsimd).dma_start(cc_in[:], cc_sb[:])
    nc.gpsimd.collective_compute(
        "AllReduce", OP.add,
        replica_groups=[list(range(N_CORES))],
        ins=[cc_in.opt()], outs=[cc_out.opt()],
    )
    g_sb = stats.tile([P, 8], F32, name=f"g_sb_{tag}")
    nc.gpsimd.dma_start(g_sb[:], cc_out[:].to_broadcast((P, 8)))
    mu = stats.tile([P, 1], F32, name=f"mu_{tag}")
    nc.vector.tensor_scalar_mul(mu[:], g_sb[:, 0:1], 1.0 / N_PGROUPS)
    ex2 = stats.tile([P, 1], F32, name=f"ex2_{tag}")
    nc.vector.tensor_scalar_mul(ex2[:], g_sb[:, 1:2], 1.0 / N_PGROUPS)
    var = stats.tile([P, 1], F32, name=f"var_{tag}")
    nc.vector.tensor_tensor(var[:], mu[:], mu[:], OP.mult)
    nc.vector.tensor_sub(var[:], ex2[:], var[:])
    nc.vector.tensor_scalar_mul(var[:], var[:], N_GLOB / (N_GLOB - 1.0))
    sd = stats.tile([P, 1], F32, name=f"sd_{tag}")
    nc.scalar.activation(sd[:], var[:], AF.Sqrt, bias=epsb[:])
    rstd = stats.tile([P, 1], F32, name=f"rstd_{tag}")
    nc.vector.reciprocal(rstd[:], sd[:])
    return mu, rstd


# ---------------------------------------------------------------------------
# Host-side input preparation / output gather
# ---------------------------------------------------------------------------

def _bf16(a):
    return np.ascontiguousarray(a.astype(ml_dtypes.bfloat16))


def _f32(a):
    return np.ascontiguousarray(a.astype(np.float32))


def _prep_shared(x, W2_w, W2_b, fc_w, fc_b, proj_w, proj_b, ln1_w, ln1_b):
    """Inputs identical on every core."""
    xqt = _bf16(x.reshape(B, 4, 512, 16, 128).transpose(0, 1, 4, 3, 2))
    w2 = _bf16(W2_w.reshape(16, 128, 16, 128).transpose(0, 3, 2, 1))
    fc_scaled = fc_w * ln1_w[None, :]
    fct = _bf16(fc_scaled.reshape(64, 128, 16, 128).transpose(0, 3, 2, 1))
    kw = _f32((fc_w @ ln1_w).reshape(64, 128).T)
    kbf = _f32((fc_w @ ln1_b + fc_b).reshape(64, 128).T)
    projt = _bf16(proj_w.reshape(16, 128, 64, 128).transpose(0, 3, 2, 1))
    projbt = _f32(proj_b.reshape(16, 128).T)
    return {"xq": xqt, "w2": w2, "fc": fct, "kw": kw, "kbf": kbf,
            "proj": projt, "projb": projbt}


def _prep_core_inputs(c, shared, x, W1_w, W1_b, W2_b, ln1_w, ln1_b, ln2_w, ln2_b):
    r0 = 256 * c
    wqk = np.concatenate([W1_w[r0:r0 + 256] * SM_SCALE,
                          W1_w[D + r0:D + r0 + 256]], axis=0)
    w1qk = _bf16(wqk.T.reshape(16, 128, 512).transpose(1, 0, 2))
    bqk = np.concatenate([W1_b[r0:r0 + 256] * SM_SCALE,
                          W1_b[D + r0:D + r0 + 256]])
    b1qk = _f32(np.ascontiguousarray(np.broadcast_to(bqk[None, :], (P, 512))))
    wv = W1_w[2 * D + r0:2 * D + r0 + 256]
    w1v = _bf16(wv.T.reshape(16, 128, 256).transpose(1, 0, 2))
    b1v = _f32(W1_b[2 * D + r0:2 * D + r0 + 256].reshape(2, 128).T)
    # residual rows (x + W2_b)^T  [n_in, n_out, i],  i = b*256 + r
    xs = x[:, r0:r0 + 256, :] + W2_b[None, None, :]
    xres = _f32(xs.transpose(2, 0, 1).reshape(16, 128, 1024).transpose(1, 0, 2))
    vec = lambda v: _f32(v.reshape(16, 128).T)
    d = {"w1qk": w1qk, "b1qk": b1qk, "w1v": w1v, "b1v": b1v, "xres": xres,
         "lnw1": vec(ln1_w), "lnb1": vec(ln1_b),
         "lnw2": vec(ln2_w), "lnb2": vec(ln2_b)}
    d.update(shared)
    return d


_NC_CACHE = None


def kernel(x, W1_w, W1_b, W2_w, W2_b, fc_w, fc_b, proj_w, proj_b,
           ln1_w, ln1_b, ln2_w, ln2_b):
    global _NC_CACHE, LAST_RESULT
    if TRACE:
        _register_ntff_hook()
    x = np.asarray(x, np.float32)
    if _NC_CACHE is None:
        _NC_CACHE = build_program()
    nc = _NC_CACHE
    shared = _prep_shared(x, np.asarray(W2_w), np.asarray(W2_b), np.asarray(fc_w),
                          np.asarray(fc_b), np.asarray(proj_w), np.asarray(proj_b),
                          np.asarray(ln1_w), np.asarray(ln1_b))
    in_maps = [
        _prep_core_inputs(c, shared, x, np.asarray(W1_w), np.asarray(W1_b),
                          np.asarray(W2_b), np.asarray(ln1_w), np.asarray(ln1_b),
                          np.asarray(ln2_w), np.asarray(ln2_b))
        for c in range(N_CORES)
    ]
    res = bass_utils.run_bass_kernel_spmd(
        nc, in_maps, core_ids=list(range(N_CORES)), trace=TRACE,
    )
    LAST_RESULT = res
    out = np.empty((B, S, D), np.float32)
    for c in range(N_CORES):
        yt = res.results[c]["y"]                    # [128 n_in, 16 n_out, 1024 i]
        blk = yt.reshape(128, 16, 4, 256).transpose(2, 3, 1, 0).reshape(4, 256, D)
        out[:, 256 * c:256 * (c + 1), :] = blk
    return out



# revision 6
# speedup vs baseline: 1.0131x; 1.0131x over previous
"""Trainium2 Bass kernel for nn_Block_58497454571919 (dense transformer block).

Reference semantics (B=4, S=2048, D=2048, H=16, Dh=128, DFF=8192):
  X = x @ W1.T + b1 ; Q,K,V = split(X)
  per (b,h): scores[d,e] = sum_s Q[b,s,hd]K[b,s,he] / sqrt(S)  (feature-attention)
             w = softmax(scores, axis=e);  out[d,s] = sum_e w[d,e] V[b,s,he]
  attn_pre[b, h*128+d, s] = out[d,s]   (raw memory reshape)
  a = attn_pre @ W2.T + b2 ; t1 = a + x ; x1 = global_scalar_LN(t1, lnw1, lnb1)
  m = gelu_tanh(x1 @ fc.T + fcb) @ proj.T + projb ; t2 = m + x1
  y = global_scalar_LN(t2, lnw2, lnb2)

Distribution over 8 cores: core c owns heads {2c, 2c+1} == output rows
[256c, 256c+256) of every batch. The QKV projection for those heads needs all
tokens (full x); W2/LN/FFN are row-parallel on the core's 4*256=1024 rows.
The only cross-core data dependency is the global-scalar LayerNorm mean/var:
two tiny AllReduces of (sum, sumsq).

LN1 is algebraically deferred so its AllReduce overlaps the FFN1 matmuls:
ln1_w is folded into fc on the host, FFN1 contracts the *unnormalized*
residual t1, and the normalization enters through the gelu activation's
per-partition scale (rstd) and bias (kbf - mu*rstd*kw, with kw/kbf host
matvecs of fc against ln1_w/ln1_b).

On-device layouts (all "transposed" so no device transposes are needed):
  QK[b]   [128 s_in, 16 s_out, 512 (q 256|k 256)] bf16
  VT[b]   [128 vf_in, 2 head, 2048 s]             bf16
  attnT   [128 s_in, 16 s_out, 1024 i]            bf16   i = b*256 + hl*128 + d
  t1b     [128 n_in, 16 n_out, 1024 i]            bf16

FFN processes both 512-token halves per weight tile so fc/proj stream from
HBM once (not twice); FFN1 drains raw pre-activations to hT and applies
gelu in place once the LN1 AllReduce lands; x1' is recomputed from t1b by
DVE in FFN2 (x1' = s1*t1b + c1, with proj_b folded into c1).
"""
import math
import os
import sys
import types

import numpy as np
import ml_dtypes

import concourse.bass as bass
import concourse.bacc as bacc
import concourse.mybir as mybir
import concourse.tile as tile
from concourse import bass_utils
from concourse.masks import make_identity

F32 = mybir.dt.float32
BF16 = mybir.dt.bfloat16
AF = mybir.ActivationFunctionType
OP = mybir.AluOpType

N_CORES = 8
B, S, D, H, DH, DFF = 4, 2048, 2048, 16, 128, 8192
P = 128
EPS = 1e-12
SM_SCALE = 1.0 / math.sqrt(S)
N_GLOB = float(B * S * D)          # 16777216 elements in each layernorm
N_PGROUPS = 1024.0                 # 8 cores * 128 partitions

TRACE = False          # set by test.py to capture an NTFF profile
LAST_RESULT = None     # BassKernelResults stash for test.py


def _register_ntff_hook():
    """The agent image's antenv lacks axon_hooks; inject it so trace=True works."""
    if "antenv.axon_hooks" in sys.modules:
        return
    mod = types.ModuleType("antenv.axon_hooks")
    mod._hook = None
    mod.set_axon_ntff_profile_hook = lambda h: setattr(mod, "_hook", h)
    mod.get_axon_ntff_profile_hook = lambda: mod._hook
    sys.modules["antenv.axon_hooks"] = mod
    import antenv

    antenv.axon_hooks = mod
    try:
        from trn_agent_boot.trn_boot import _ntff_profile_via_ctypes

        mod.set_axon_ntff_profile_hook(
            _ntff_profile_via_ctypes("/opt/axon/libaxon_pjrt.so")
        )
    except Exception:
        pass


def build_program():
    nc = bacc.Bacc("TRN2", target_bir_lowering=False, debug=False, num_devices=N_CORES)

    def din(name, shape, dtype):
        return nc.dram_tensor(name, shape, dtype, kind="ExternalInput").ap()

    ins = {
        "xq": din("xq", [B, 4, P, 16, 512], BF16),     # x^T tiles [b, sb, d_in, d_out, s]
        "w1qk": din("w1qk", [P, 16, 512], BF16),       # [d_in, d_out, (q|k) feat]
        "b1qk": din("b1qk", [P, 512], F32),            # replicated over partitions
        "w1v": din("w1v", [P, 16, 256], BF16),         # [d_in, d_out, vfeat]
        "b1v": din("b1v", [P, 2], F32),                # [vf_in, head]
        "w2": din("w2", [16, P, 16, 128], BF16),       # [n_blk, s_in, s_out, n]
        "xres": din("xres", [P, 16, 1024], BF16),      # (x + b2)^T slice [n_in, n_out, i]
        "fc": din("fc", [64, P, 16, 128], BF16),       # ln1_w-scaled fc^T tiles
        "kw": din("kw", [P, 64], F32),                 # fc @ ln1_w     [f_in, f_blk]
        "kbf": din("kbf", [P, 64], F32),               # fc @ ln1_b + fc_b
        "proj": din("proj", [16, P, 64, 128], BF16),   # [n_blk, f_in, f_out, n]
        "projb": din("projb", [P, 16], F32),           # [n_in, n_out]
        "lnw1": din("lnw1", [P, 16], F32),
        "lnb1": din("lnb1", [P, 16], F32),
        "lnw2": din("lnw2", [P, 16], F32),
        "lnb2": din("lnb2", [P, 16], F32),
    }
    y_out = nc.dram_tensor("y", [P, 16, 1024], F32, kind="ExternalOutput").ap()

    with tile.TileContext(nc) as tc:
        _emit(nc, tc, ins, y_out)
    nc.compile()
    return nc


def _emit(nc, tc, I, y_out):
    with (
        tc.tile_pool(name="consts", bufs=1) as consts,
        tc.tile_pool(name="stats", bufs=1) as stats,
        tc.tile_pool(name="small", bufs=3) as small,
        tc.tile_pool(name="dram", bufs=1, space="DRAM") as dram,
        tc.tile_pool(name="ps_red", bufs=1, space="PSUM") as ps_red,
        tc.tile_pool(name="t1pool", bufs=1) as t1_pool,
    ):
        # t1b first so its pool exists before anything else writes it
        t1b = t1_pool.tile([P, 16, 1024], BF16, name="t1b")
        stats1 = stats.tile([P, 32, 6], F32, name="stats1")
        stats2 = stats.tile([P, 32, 6], F32, name="stats2")
        t2_dram = dram.tile([P, 16, 1024], BF16, name="t2_dram")

        with (
            tc.tile_pool(name="attn", bufs=1) as attn_pool,
            tc.tile_pool(name="ps_big", bufs=4, space="PSUM") as ps_big,
            tc.tile_pool(name="ps_sm", bufs=3, space="PSUM") as ps_sm,
        ):
            attnT = attn_pool.tile([P, 16, 1024], BF16, name="attnT")

            with (
                tc.tile_pool(name="w1pool", bufs=1) as w1_pool,
                tc.tile_pool(name="xq", bufs=2) as xq_pool,
                tc.tile_pool(name="qkpool", bufs=2) as qk_pool,
            ):
                # critical-path DMAs first: QKV weights + first x tiles
                w1qk_sb = w1_pool.tile([P, 16, 512], BF16, name="w1qk_sb")
                for q in range(4):
                    nc.sync.dma_start(w1qk_sb[:, 4 * q:4 * q + 4, :],
                                      I["w1qk"][:, 4 * q:4 * q + 4, :])
                w1v_sb = w1_pool.tile([P, 16, 256], BF16, name="w1v_sb")
                nc.sync.dma_start(w1v_sb[:], I["w1v"][:])
                b1qk_sb = w1_pool.tile([P, 512], F32, name="b1qk_sb")
                nc.sync.dma_start(b1qk_sb[:], I["b1qk"][:])
                b1v_sb = w1_pool.tile([P, 2], F32, name="b1v_sb")
                nc.sync.dma_start(b1v_sb[:], I["b1v"][:])
                ident = consts.tile([P, P], BF16, name="ident")
                make_identity(nc, ident[:])
                ones = consts.tile([P, 1], F32, name="ones")
                nc.vector.memset(ones[:], 1.0)
                epsb = consts.tile([P, 1], F32, name="epsb")
                nc.vector.memset(epsb[:], EPS)
                # PE warmup: dummy matmuls (no DMA deps) so the HAM clock
                # gate is released before the first real chain arrives.
                pwarm = ps_sm.tile([P, P], F32, name="pwarm", tag="pssm")
                for _ in range(80):
                    nc.tensor.matmul(pwarm[:], ident[:], ident[:],
                                     start=True, stop=True)

                def load_consts():
                    tiles = {}
                    for nm in ("kw", "kbf", "projb", "lnw1", "lnb1", "lnw2", "lnb2"):
                        t = consts.tile(list(I[nm].shape), F32, name=f"{nm}_sb")
                        nc.sync.dma_start(t[:], I[nm][:])
                        tiles[nm] = t
                    return tiles

                def phase12(b):
                    QK = qk_pool.tile([P, 16, 512], BF16, name="QK", tag="QK")
                    VT = qk_pool.tile([P, 2, S], BF16, name="VT", tag="VT")
                    for sb in range(4):
                        xt = xq_pool.tile([P, 16, 512], BF16, name="xt", tag="xt")
                        if b == 0 and sb == 0:
                            for q in range(4):
                                eng = nc.gpsimd if q == 0 else nc.sync
                                eng.dma_start(xt[:, 4 * q:4 * q + 4, :],
                                              I["xq"][b, sb, :, 4 * q:4 * q + 4, :])
                        else:
                            nc.sync.dma_start(xt[:, 0:8, :], I["xq"][b, sb, :, 0:8, :])
                            nc.sync.dma_start(xt[:, 8:16, :], I["xq"][b, sb, :, 8:16, :])
                        for ss in range(4):
                            pqk = ps_big.tile([P, 512], F32, name="pqk", tag="psbig")
                            for do in range(16):
                                nc.tensor.matmul(
                                    pqk[:], xt[:, do, ss * 128:(ss + 1) * 128],
                                    w1qk_sb[:, do, :], start=(do == 0), stop=(do == 15),
                                )
                            nc.vector.tensor_tensor(
                                QK[:, sb * 4 + ss, :], pqk[:], b1qk_sb[:], OP.add)
                        for vo in range(2):
                            pv = ps_big.tile([P, 512], F32, name="pv", tag="psbig")
                            for do in range(16):
                                nc.tensor.matmul(
                                    pv[:], w1v_sb[:, do, vo * 128:(vo + 1) * 128],
                                    xt[:, do, :], start=(do == 0), stop=(do == 15),
                                )
                            nc.vector.tensor_scalar(
                                VT[:, vo, sb * 512:(sb + 1) * 512], pv[:],
                                b1v_sb[:, vo:vo + 1], None, OP.add)
                    # both heads' score chains first so the PE is never
                    # queued behind a softmax-gated transpose
                    pscores = []
                    for hl in range(2):
                        pscore = ps_sm.tile([P, P], F32, name="pscore", tag="pssm")
                        for so in range(16):
                            nc.tensor.matmul(
                                pscore[:], QK[:, so, hl * 128:(hl + 1) * 128],
                                QK[:, so, 256 + hl * 128:256 + (hl + 1) * 128],
                                start=(so == 0), stop=(so == 15),
                            )
                        pscores.append(pscore)
                    # 1/sqrt(S) is folded into the Q weights on the host,
                    # so psum scores are pre-scaled: exp(x - max) directly.
                    wnorms = []
                    for hl in range(2):
                        negmax = small.tile([P, 1], F32, name="negmax", tag="negmax")
                        nc.vector.reduce_max(negmax[:], pscores[hl][:],
                                             axis=mybir.AxisListType.X, negate=True)
                        wexp = small.tile([P, P], F32, name="wexp", tag="wexp")
                        rowsum = small.tile([P, 1], F32, name="rowsum", tag="rowsum")
                        nc.scalar.activation(wexp[:], pscores[hl][:], AF.Exp,
                                             bias=negmax[:], scale=1.0,
                                             accum_out=rowsum[:])
                        rinv = small.tile([P, 1], F32, name="rinv", tag="rinv")
                        nc.vector.reciprocal(rinv[:], rowsum[:])
                        wnorm = small.tile([P, P], BF16, name="wnorm", tag="wnorm")
                        nc.vector.tensor_scalar_mul(wnorm[:], wexp[:], rinv[:])
                        wnorms.append(wnorm)
                    for hl in range(2):
                        pwt = ps_sm.tile([P, P], BF16, name="pwt", tag="pssm")
                        nc.tensor.transpose(pwt[:], wnorms[hl][:], ident[:])
                        wT = small.tile([P, P], BF16, name="wT", tag="wT")
                        nc.vector.tensor_copy(wT[:], pwt[:])
                        for so in range(16):
                            pat = ps_sm.tile([P, P], F32, name="pat", tag="pssm")
                            nc.tensor.matmul(
                                pat[:], VT[:, hl, so * 128:(so + 1) * 128], wT[:],
                                start=True, stop=True,
                            )
                            nc.vector.tensor_copy(
                                attnT[:, so, b * 256 + hl * 128:b * 256 + (hl + 1) * 128],
                                pat[:])

                def w2_half(bp, w2_pool):
                    for nb in range(16):
                        w2t = w2_pool.tile([P, 16, 128], BF16, name="w2t", tag="w2t")
                        nc.sync.dma_start(w2t[:], I["w2"][nb])
                        pw2 = ps_big.tile([P, 512], F32, name="pw2", tag="psbig")
                        for so in range(16):
                            nc.tensor.matmul(
                                pw2[:], w2t[:, so, :],
                                attnT[:, so, bp * 512:(bp + 1) * 512],
                                start=(so == 0), stop=(so == 15),
                            )
                        xr = w2_pool.tile([P, 512], BF16, name="xr", tag="xr")
                        nc.sync.dma_start(xr[:], I["xres"][:, nb, bp * 512:(bp + 1) * 512])
                        t1s = t1b[:, nb, bp * 512:(bp + 1) * 512]
                        nc.vector.tensor_tensor(t1s, pw2[:], xr[:], OP.add)
                        nc.vector.bn_stats(stats1[:, nb * 2 + bp, :], t1s)

                with tc.tile_pool(name="w2pool", bufs=3) as w2_pool:
                    phase12(0)
                    C = load_consts()
                    phase12(1)
                    w2_half(0, w2_pool)
                    phase12(2)
                    phase12(3)
                    dum1 = stats.tile([P, 1], F32, name="dum1")
                    nc.scalar.activation(dum1[:], epsb[:], AF.Sqrt)
                    w2_half(1, w2_pool)

        # ---- LN1 scalars (AllReduce overlaps the FFN1 matmuls below) ----
        mu1, rstd1 = _ln_allreduce(nc, stats, dram, ps_red, ones, epsb, stats1, "ln1")
        murstd1 = stats.tile([P, 1], F32, name="murstd1")
        nc.vector.tensor_tensor(murstd1[:], mu1[:], rstd1[:], OP.mult)
        # gelu bias: kbf - mu*rstd*kw   [128, 64]
        gbias = stats.tile([P, 64], F32, name="gbias")
        nc.vector.tensor_scalar_mul(gbias[:], C["kw"][:], murstd1[:])
        nc.vector.tensor_sub(gbias[:], C["kbf"][:], gbias[:])
        # x1' scalars: x1' = s1*t1b + c1 (recomputed on the fly in FFN2)
        s1 = stats.tile([P, 16], F32, name="s1")
        nc.vector.tensor_scalar_mul(s1[:], C["lnw1"][:], rstd1[:])
        c1 = stats.tile([P, 16], F32, name="c1")
        nc.vector.tensor_scalar_mul(c1[:], s1[:], mu1[:])
        nc.vector.tensor_sub(c1[:], C["lnb1"][:], c1[:])
        nc.vector.tensor_add(c1[:], c1[:], C["projb"][:])
        # ============ FFN (both halves per weight tile; fc/proj loaded once) ====
        with tc.tile_pool(name="hpool", bufs=1) as h_pool:
            hT = h_pool.tile([P, 64, 1024], BF16, name="hT")
            with (
                tc.tile_pool(name="fcpool", bufs=3) as fc_pool,
                tc.tile_pool(name="ps_h", bufs=6, space="PSUM") as ps_h,
            ):
                for fb in range(64):
                    fct = fc_pool.tile([P, 16, 128], BF16, name="fct", tag="fct")
                    nc.sync.dma_start(fct[:], I["fc"][fb])
                    for ch in range(2):
                        ph = ps_h.tile([P, 512], F32, name="ph", tag="psh")
                        for do in range(16):
                            nc.tensor.matmul(
                                ph[:], fct[:, do, :],
                                t1b[:, do, ch * 512:(ch + 1) * 512],
                                start=(do == 0), stop=(do == 15),
                            )
                        # drain raw pre-activation; gelu applied in place later
                        # so the PE never backpressures on the LN1 AllReduce
                        nc.vector.tensor_copy(hT[:, fb, ch * 512:(ch + 1) * 512],
                                              ph[:])
                    # gelu in place once both halves drained (gated on rstd1)
                    nc.scalar.activation(hT[:, fb, :], hT[:, fb, :],
                                         AF.Gelu_apprx_tanh,
                                         bias=gbias[:, fb:fb + 1], scale=rstd1[:])
            with (
                tc.tile_pool(name="projpool", bufs=2) as proj_pool,
                tc.tile_pool(name="respool", bufs=3) as res_pool,
                tc.tile_pool(name="ps_m", bufs=4, space="PSUM") as ps_m,
            ):
                for nb in range(16):
                    pjt = proj_pool.tile([P, 64, 128], BF16, name="pjt", tag="pjt")
                    nc.sync.dma_start(pjt[:], I["proj"][nb])
                    for ch in range(2):
                        pm = ps_m.tile([P, 512], F32, name="pm", tag="psm")
                        for fo in range(64):
                            nc.tensor.matmul(
                                pm[:], pjt[:, fo, :],
                                hT[:, fo, ch * 512:(ch + 1) * 512],
                                start=(fo == 0), stop=(fo == 63),
                            )
                        x1r = res_pool.tile([P, 512], BF16, name="x1r", tag="x1r")
                        nc.vector.tensor_scalar(
                            x1r[:], t1b[:, nb, ch * 512:(ch + 1) * 512],
                            s1[:, nb:nb + 1], c1[:, nb:nb + 1], OP.mult, OP.add)
                        t2s = res_pool.tile([P, 512], BF16, name="t2s", tag="t2s")
                        nc.vector.tensor_tensor(t2s[:], pm[:], x1r[:], OP.add)
                        nc.vector.bn_stats(stats2[:, nb * 2 + ch, :], t2s[:])
                        nc.gpsimd.dma_start(
                            t2_dram[:, nb, ch * 512:(ch + 1) * 512], t2s[:])

        # ============ LN2 (AllReduce) -> output ============
        dum2 = stats.tile([P, 1], F32, name="dum2")
        nc.scalar.activation(dum2[:], epsb[:], AF.Sqrt)
        with tc.tile_pool(name="outpool", bufs=1) as out_pool:
            t2r = out_pool.tile([P, 16, 1024], BF16, name="t2r")
            nc.sync.dma_start(t2r[:], t2_dram[:])
            mu2, rstd2 = _ln_allreduce(nc, stats, dram, ps_red, ones, epsb, stats2, "ln2", cc_in_sync=True)
            s2 = stats.tile([P, 16], F32, name="s2")
            nc.vector.tensor_scalar_mul(s2[:], C["lnw2"][:], rstd2[:])
            c2 = stats.tile([P, 16], F32, name="c2")
            nc.vector.tensor_scalar_mul(c2[:], s2[:], mu2[:])
            nc.vector.tensor_sub(c2[:], C["lnb2"][:], c2[:])
            ys = out_pool.tile([P, 16, 1024], F32, name="ys")
            for nb in range(16):
                if nb % 2 == 0:
                    nc.vector.tensor_scalar(
                        ys[:, nb, :], t2r[:, nb, :],
                        s2[:, nb:nb + 1], c2[:, nb:nb + 1], OP.mult, OP.add)
                else:
                    nc.scalar.activation(
                        ys[:, nb, :], t2r[:, nb, :], AF.Identity,
                        bias=c2[:, nb:nb + 1], scale=s2[:, nb:nb + 1])
                nc.sync.dma_start(y_out[:, nb:nb + 1, :], ys[:, nb:nb + 1, :])


def _ln_allreduce(nc, stats, dram, ps_red, ones, epsb, stats_t, tag, cc_in_sync=False):
    """bn_stats tiles -> global scalar mean + rstd via cross-core AllReduce."""
    mv = stats.tile([P, 2], F32, name=f"mv_{tag}")
    nc.vector.bn_aggr(mv[:], stats_t[:])
    # red_in[:,0] = mean_p ; red_in[:,1] = meansq_p = var_p + mean_p^2
    red_in = stats.tile([P, 2], F32, name=f"red_in_{tag}")
    nc.vector.tensor_copy(red_in[:, 0:1], mv[:, 0:1])
    nc.vector.tensor_tensor(red_in[:, 1:2], mv[:, 0:1], mv[:, 0:1], OP.mult)
    nc.vector.tensor_tensor(red_in[:, 1:2], red_in[:, 1:2], mv[:, 1:2], OP.add)
    pred = ps_red.tile([1, 2], F32, name=f"pred_{tag}", tag="psred")
    nc.tensor.matmul(pred[:], ones[:], red_in[:], start=True, stop=True)
    cc_sb = stats.tile([1, 8], F32, name=f"cc_sb_{tag}")
    nc.vector.memset(cc_sb[:], 0.0)
    nc.vector.tensor_copy(cc_sb[:, 0:2], pred[:])
    cc_in = dram.tile([1, 8], F32, name=f"cc_in_{tag}")
    cc_out = dram.tile([1, 8], F32, name=f"cc_out_{tag}", addr_space="Shared")
    (nc.sync if cc_in_sync else nc.gpsimd).dma_start(cc_in[:], cc_sb[:])
    nc.gpsimd.collective_compute(
        "AllReduce", OP.add,
        replica_groups=[list(range(N_CORES))],
        ins=[cc_in.opt()], outs=[cc_out.opt()],
    )
    g_sb = stats.tile([P, 8], F32, name=f"g_sb_{tag}")
    nc.gpsimd.dma_start(g_sb[:], cc_out[:].to_broadcast((P, 8)))
    mu = stats.tile([P, 1], F32, name=f"mu_{tag}")
    nc.vector.tensor_scalar_mul(mu[:], g_sb[:, 0:1], 1.0 / N_PGROUPS)
    ex2 = stats.tile([P, 1], F32, name=f"ex2_{tag}")
    nc.vector.tensor_scalar_mul(ex2[:], g_sb[:, 1:2], 1.0 / N_PGROUPS)
    var = stats.tile([P, 1], F32, name=f"var_{tag}")
    nc.vector.tensor_tensor(var[:], mu[:], mu[:], OP.mult)
    nc.vector.tensor_sub(var[:], ex2[:], var[:])
    nc.vector.tensor_scalar_mul(var[:], var[:], N_GLOB / (N_GLOB - 1.0))
    sd = stats.tile([P, 1], F32, name=f"sd_{tag}")
    nc.scalar.activation(sd[:], var[:], AF.Sqrt, bias=epsb[:])
    rstd = stats.tile([P, 1], F32, name=f"rstd_{tag}")
    nc.vector.reciprocal(rstd[:], sd[:])
    return mu, rstd


# ---------------------------------------------------------------------------
# Host-side input preparation / output gather
# ---------------------------------------------------------------------------

def _bf16(a):
    return np.ascontiguousarray(a.astype(ml_dtypes.bfloat16))


def _f32(a):
    return np.ascontiguousarray(a.astype(np.float32))


def _prep_shared(x, W2_w, W2_b, fc_w, fc_b, proj_w, proj_b, ln1_w, ln1_b):
    """Inputs identical on every core."""
    xqt = _bf16(x.reshape(B, 4, 512, 16, 128).transpose(0, 1, 4, 3, 2))
    w2 = _bf16(W2_w.reshape(16, 128, 16, 128).transpose(0, 3, 2, 1))
    fc_scaled = fc_w * ln1_w[None, :]
    fct = _bf16(fc_scaled.reshape(64, 128, 16, 128).transpose(0, 3, 2, 1))
    kw = _f32((fc_w @ ln1_w).reshape(64, 128).T)
    kbf = _f32((fc_w @ ln1_b + fc_b).reshape(64, 128).T)
    projt = _bf16(proj_w.reshape(16, 128, 64, 128).transpose(0, 3, 2, 1))
    projbt = _f32(proj_b.reshape(16, 128).T)
    return {"xq": xqt, "w2": w2, "fc": fct, "kw": kw, "kbf": kbf,
            "proj": projt, "projb": projbt}


def _prep_core_inputs(c, shared, x, W1_w, W1_b, W2_b, ln1_w, ln1_b, ln2_w, ln2_b):
    r0 = 256 * c
    wqk = np.concatenate([W1_w[r0:r0 + 256] * SM_SCALE,
                          W1_w[D + r0:D + r0 + 256]], axis=0)
    w1qk = _bf16(wqk.T.reshape(16, 128, 512).transpose(1, 0, 2))
    bqk = np.concatenate([W1_b[r0:r0 + 256] * SM_SCALE,
                          W1_b[D + r0:D + r0 + 256]])
    b1qk = _f32(np.ascontiguousarray(np.broadcast_to(bqk[None, :], (P, 512))))
    wv = W1_w[2 * D + r0:2 * D + r0 + 256]
    w1v = _bf16(wv.T.reshape(16, 128, 256).transpose(1, 0, 2))
    b1v = _f32(W1_b[2 * D + r0:2 * D + r0 + 256].reshape(2, 128).T)
    # residual rows (x + W2_b)^T  [n_in, n_out, i],  i = b*256 + r
    xs = x[:, r0:r0 + 256, :] + W2_b[None, None, :]
    xres = _bf16(xs.transpose(2, 0, 1).reshape(16, 128, 1024).transpose(1, 0, 2))
    vec = lambda v: _f32(v.reshape(16, 128).T)
    d = {"w1qk": w1qk, "b1qk": b1qk, "w1v": w1v, "b1v": b1v, "xres": xres,
         "lnw1": vec(ln1_w), "lnb1": vec(ln1_b),
         "lnw2": vec(ln2_w), "lnb2": vec(ln2_b)}
    d.update(shared)
    return d


_NC_CACHE = None


def kernel(x, W1_w, W1_b, W2_w, W2_b, fc_w, fc_b, proj_w, proj_b,
           ln1_w, ln1_b, ln2_w, ln2_b):
    global _NC_CACHE, LAST_RESULT
    if TRACE:
        _register_ntff_hook()
    x = np.asarray(x, np.float32)
    if _NC_CACHE is None:
        _NC_CACHE = build_program()
    nc = _NC_CACHE
    shared = _prep_shared(x, np.asarray(W2_w), np.asarray(W2_b), np.asarray(fc_w),
                          np.asarray(fc_b), np.asarray(proj_w), np.asarray(proj_b),
                          np.asarray(ln1_w), np.asarray(ln1_b))
    in_maps = [
        _prep_core_inputs(c, shared, x, np.asarray(W1_w), np.asarray(W1_b),
                          np.asarray(W2_b), np.asarray(ln1_w), np.asarray(ln1_b),
                          np.asarray(ln2_w), np.asarray(ln2_b))
        for c in range(N_CORES)
    ]
    res = bass_utils.run_bass_kernel_spmd(
        nc, in_maps, core_ids=list(range(N_CORES)), trace=TRACE,
    )
    LAST_RESULT = res
    out = np.empty((B, S, D), np.float32)
    for c in range(N_CORES):
        yt = res.results[c]["y"]                    # [128 n_in, 16 n_out, 1024 i]
        blk = yt.reshape(128, 16, 4, 256).transpose(2, 3, 1, 0).reshape(4, 256, D)
        out[:, 256 * c:256 * (c + 1), :] = blk
    return out



# revision 12
# speedup vs baseline: 1.0196x; 1.0065x over previous
"""Trainium2 Bass kernel for nn_Block_58497454571919 (dense transformer block).

Reference semantics (B=4, S=2048, D=2048, H=16, Dh=128, DFF=8192):
  X = x @ W1.T + b1 ; Q,K,V = split(X)
  per (b,h): scores[d,e] = sum_s Q[b,s,hd]K[b,s,he] / sqrt(S)  (feature-attention)
             w = softmax(scores, axis=e);  out[d,s] = sum_e w[d,e] V[b,s,he]
  attn_pre[b, h*128+d, s] = out[d,s]   (raw memory reshape)
  a = attn_pre @ W2.T + b2 ; t1 = a + x ; x1 = global_scalar_LN(t1, lnw1, lnb1)
  m = gelu_tanh(x1 @ fc.T + fcb) @ proj.T + projb ; t2 = m + x1
  y = global_scalar_LN(t2, lnw2, lnb2)

Distribution over 8 cores: core c owns heads {2c, 2c+1} == output rows
[256c, 256c+256) of every batch. The QKV projection for those heads needs all
tokens (full x); W2/LN/FFN are row-parallel on the core's 4*256=1024 rows.
The only cross-core data dependency is the global-scalar LayerNorm mean/var:
two tiny AllReduces of (sum, sumsq).

LN1 is algebraically deferred so its AllReduce overlaps the FFN1 matmuls:
ln1_w is folded into fc on the host, FFN1 contracts the *unnormalized*
residual t1, and the normalization enters through the gelu activation's
per-partition scale (rstd) and bias (kbf - mu*rstd*kw, with kw/kbf host
matvecs of fc against ln1_w/ln1_b).

On-device layouts (all "transposed" so no device transposes are needed):
  QK[b]   [128 s_in, 16 s_out, 512 (q 256|k 256)] bf16
  VT[b]   [128 vf_in, 2 head, 2048 s]             bf16
  attnT   [128 s_in, 16 s_out, 1024 i]            bf16   i = b*256 + hl*128 + d
  t1b     [128 n_in, 16 n_out, 1024 i]            bf16

FFN processes both 512-token halves per weight tile so fc/proj stream from
HBM once (not twice); FFN1 drains raw pre-activations to hT and applies
gelu in place once the LN1 AllReduce lands; x1' is recomputed from t1b by
DVE in FFN2 (x1' = s1*t1b + c1, with proj_b folded into c1).
"""
import math
import os
import sys
import types

import numpy as np
import ml_dtypes

import concourse.bass as bass
import concourse.bacc as bacc
import concourse.mybir as mybir
import concourse.tile as tile
from concourse import bass_utils
from concourse.masks import make_identity

F32 = mybir.dt.float32
BF16 = mybir.dt.bfloat16
AF = mybir.ActivationFunctionType
OP = mybir.AluOpType

N_CORES = 8
B, S, D, H, DH, DFF = 4, 2048, 2048, 16, 128, 8192
P = 128
EPS = 1e-12
SM_SCALE = 1.0 / math.sqrt(S)
N_GLOB = float(B * S * D)          # 16777216 elements in each layernorm
N_PGROUPS = 1024.0                 # 8 cores * 128 partitions

TRACE = False          # set by test.py to capture an NTFF profile
LAST_RESULT = None     # BassKernelResults stash for test.py


def _register_ntff_hook():
    """The agent image's antenv lacks axon_hooks; inject it so trace=True works."""
    if "antenv.axon_hooks" in sys.modules:
        return
    mod = types.ModuleType("antenv.axon_hooks")
    mod._hook = None
    mod.set_axon_ntff_profile_hook = lambda h: setattr(mod, "_hook", h)
    mod.get_axon_ntff_profile_hook = lambda: mod._hook
    sys.modules["antenv.axon_hooks"] = mod
    import antenv

    antenv.axon_hooks = mod
    try:
        from trn_agent_boot.trn_boot import _ntff_profile_via_ctypes

        mod.set_axon_ntff_profile_hook(
            _ntff_profile_via_ctypes("/opt/axon/libaxon_pjrt.so")
        )
    except Exception:
        pass


def build_program():
    nc = bacc.Bacc("TRN2", target_bir_lowering=False, debug=False, num_devices=N_CORES)

    def din(name, shape, dtype):
        return nc.dram_tensor(name, shape, dtype, kind="ExternalInput").ap()

    ins = {
        "xq": din("xq", [B, 4, P, 16, 512], BF16),     # x^T tiles [b, sb, d_in, d_out, s]
        "w1qk": din("w1qk", [P, 16, 512], BF16),       # [d_in, d_out, (q|k) feat]
        "b1qk": din("b1qk", [P, 512], F32),            # replicated over partitions
        "w1v": din("w1v", [P, 16, 256], BF16),         # [d_in, d_out, vfeat]
        "b1v": din("b1v", [P, 2], F32),                # [vf_in, head]
        "w2": din("w2", [16, P, 16, 128], BF16),       # [n_blk, s_in, s_out, n]
        "xres": din("xres", [P, 16, 1024], BF16),      # (x + b2)^T slice [n_in, n_out, i]
        "fc": din("fc", [64, P, 16, 128], BF16),       # ln1_w-scaled fc^T tiles
        "kw": din("kw", [P, 64], F32),                 # fc @ ln1_w     [f_in, f_blk]
        "kbf": din("kbf", [P, 64], F32),               # fc @ ln1_b + fc_b
        "proj": din("proj", [16, P, 64, 128], BF16),   # [n_blk, f_in, f_out, n]
        "projb": din("projb", [P, 16], F32),           # [n_in, n_out]
        "lnw1": din("lnw1", [P, 16], F32),
        "lnb1": din("lnb1", [P, 16], F32),
        "lnw2": din("lnw2", [P, 16], F32),
        "lnb2": din("lnb2", [P, 16], F32),
    }
    y_out = nc.dram_tensor("y", [P, 16, 1024], F32, kind="ExternalOutput").ap()

    with tile.TileContext(nc) as tc:
        _emit(nc, tc, ins, y_out)
    nc.compile()
    return nc


def _emit(nc, tc, I, y_out):
    with (
        tc.tile_pool(name="consts", bufs=1) as consts,
        tc.tile_pool(name="stats", bufs=1) as stats,
        tc.tile_pool(name="dram", bufs=1, space="DRAM") as dram,
        tc.tile_pool(name="ps_red", bufs=1, space="PSUM") as ps_red,
        tc.tile_pool(name="t1pool", bufs=1) as t1_pool,
    ):
        # t1b first so its pool exists before anything else writes it
        t1b = t1_pool.tile([P, 16, 1024], BF16, name="t1b")
        stats1 = stats.tile([P, 32, 6], F32, name="stats1")
        stats2 = stats.tile([P, 32, 6], F32, name="stats2")
        t2_dram = dram.tile([P, 16, 1024], BF16, name="t2_dram")

        with (
            tc.tile_pool(name="attn", bufs=1) as attn_pool,
            tc.tile_pool(name="small", bufs=3) as small,
            tc.tile_pool(name="ps_big", bufs=4, space="PSUM") as ps_big,
            tc.tile_pool(name="ps_sm", bufs=3, space="PSUM") as ps_sm,
        ):
            attnT = attn_pool.tile([P, 16, 1024], BF16, name="attnT")

            with (
                tc.tile_pool(name="w1pool", bufs=1) as w1_pool,
                tc.tile_pool(name="xq", bufs=2) as xq_pool,
                tc.tile_pool(name="qkpool", bufs=2) as qk_pool,
            ):
                # critical-path DMAs first: QKV weights + first x tiles
                w1qk_sb = w1_pool.tile([P, 16, 512], BF16, name="w1qk_sb")
                for q in range(4):
                    nc.sync.dma_start(w1qk_sb[:, 4 * q:4 * q + 4, :],
                                      I["w1qk"][:, 4 * q:4 * q + 4, :])
                w1v_sb = w1_pool.tile([P, 16, 256], BF16, name="w1v_sb")
                nc.sync.dma_start(w1v_sb[:], I["w1v"][:])
                b1qk_sb = w1_pool.tile([P, 512], F32, name="b1qk_sb")
                nc.sync.dma_start(b1qk_sb[:], I["b1qk"][:])
                b1v_sb = w1_pool.tile([P, 2], F32, name="b1v_sb")
                nc.sync.dma_start(b1v_sb[:], I["b1v"][:])
                ident = consts.tile([P, P], BF16, name="ident")
                make_identity(nc, ident[:])
                ones = consts.tile([P, 1], F32, name="ones")
                nc.vector.memset(ones[:], 1.0)
                epsb = consts.tile([P, 1], F32, name="epsb")
                nc.vector.memset(epsb[:], EPS)
                # PE warmup: dummy matmuls (no DMA deps) so the HAM clock
                # gate is released before the first real chain arrives.
                pwarm = ps_sm.tile([P, P], F32, name="pwarm", tag="pssm")
                for _ in range(150):
                    nc.tensor.matmul(pwarm[:], ident[:], ident[:],
                                     start=True, stop=True)

                def load_consts():
                    tiles = {}
                    for nm in ("kw", "kbf", "projb", "lnw1", "lnb1", "lnw2", "lnb2"):
                        t = consts.tile(list(I[nm].shape), F32, name=f"{nm}_sb")
                        nc.sync.dma_start(t[:], I[nm][:])
                        tiles[nm] = t
                    return tiles

                def phase12(b):
                    QK = qk_pool.tile([P, 16, 512], BF16, name="QK", tag="QK")
                    VT = qk_pool.tile([P, 2, S], BF16, name="VT", tag="VT")
                    for sb in range(4):
                        xt = xq_pool.tile([P, 16, 512], BF16, name="xt", tag="xt")
                        if b == 0 and sb == 0:
                            for q in range(4):
                                eng = nc.gpsimd if q == 0 else nc.sync
                                eng.dma_start(xt[:, 4 * q:4 * q + 4, :],
                                              I["xq"][b, sb, :, 4 * q:4 * q + 4, :])
                        else:
                            nc.sync.dma_start(xt[:, 0:8, :], I["xq"][b, sb, :, 0:8, :])
                            nc.sync.dma_start(xt[:, 8:16, :], I["xq"][b, sb, :, 8:16, :])
                        for ss in range(4):
                            pqk = ps_big.tile([P, 512], F32, name="pqk", tag="psbig")
                            for do in range(16):
                                nc.tensor.matmul(
                                    pqk[:], xt[:, do, ss * 128:(ss + 1) * 128],
                                    w1qk_sb[:, do, :], start=(do == 0), stop=(do == 15),
                                )
                            nc.vector.tensor_tensor(
                                QK[:, sb * 4 + ss, :], pqk[:], b1qk_sb[:], OP.add)
                        for vo in range(2):
                            pv = ps_big.tile([P, 512], F32, name="pv", tag="psbig")
                            for do in range(16):
                                nc.tensor.matmul(
                                    pv[:], w1v_sb[:, do, vo * 128:(vo + 1) * 128],
                                    xt[:, do, :], start=(do == 0), stop=(do == 15),
                                )
                            nc.vector.tensor_scalar(
                                VT[:, vo, sb * 512:(sb + 1) * 512], pv[:],
                                b1v_sb[:, vo:vo + 1], None, OP.add)
                    # both heads' score chains first so the PE is never
                    # queued behind a softmax-gated transpose
                    pscores = []
                    for hl in range(2):
                        pscore = ps_sm.tile([P, P], F32, name="pscore", tag="pssm")
                        for so in range(16):
                            nc.tensor.matmul(
                                pscore[:], QK[:, so, hl * 128:(hl + 1) * 128],
                                QK[:, so, 256 + hl * 128:256 + (hl + 1) * 128],
                                start=(so == 0), stop=(so == 15),
                            )
                        pscores.append(pscore)
                    # 1/sqrt(S) is folded into the Q weights on the host,
                    # so psum scores are pre-scaled: exp(x - max) directly.
                    wnorms = []
                    for hl in range(2):
                        negmax = small.tile([P, 1], F32, name="negmax", tag="negmax")
                        nc.vector.reduce_max(negmax[:], pscores[hl][:],
                                             axis=mybir.AxisListType.X, negate=True)
                        wexp = small.tile([P, P], F32, name="wexp", tag="wexp")
                        rowsum = small.tile([P, 1], F32, name="rowsum", tag="rowsum")
                        nc.scalar.activation(wexp[:], pscores[hl][:], AF.Exp,
                                             bias=negmax[:], scale=1.0,
                                             accum_out=rowsum[:])
                        rinv = small.tile([P, 1], F32, name="rinv", tag="rinv")
                        nc.vector.reciprocal(rinv[:], rowsum[:])
                        wnorm = small.tile([P, P], BF16, name="wnorm", tag="wnorm")
                        nc.vector.tensor_scalar_mul(wnorm[:], wexp[:], rinv[:])
                        wnorms.append(wnorm)
                    for hl in range(2):
                        pwt = ps_sm.tile([P, P], BF16, name="pwt", tag="pssm")
                        nc.tensor.transpose(pwt[:], wnorms[hl][:], ident[:])
                        wT = small.tile([P, P], BF16, name="wT", tag="wT")
                        nc.vector.tensor_copy(wT[:], pwt[:])
                        for so in range(16):
                            pat = ps_sm.tile([P, P], F32, name="pat", tag="pssm")
                            nc.tensor.matmul(
                                pat[:], VT[:, hl, so * 128:(so + 1) * 128], wT[:],
                                start=True, stop=True,
                            )
                            nc.vector.tensor_copy(
                                attnT[:, so, b * 256 + hl * 128:b * 256 + (hl + 1) * 128],
                                pat[:])

                def w2_half(bp, w2_pool):
                    for nb in range(16):
                        w2t = w2_pool.tile([P, 16, 128], BF16, name="w2t", tag="w2t")
                        nc.sync.dma_start(w2t[:], I["w2"][nb])
                        pw2 = ps_big.tile([P, 512], F32, name="pw2", tag="psbig")
                        for so in range(16):
                            nc.tensor.matmul(
                                pw2[:], w2t[:, so, :],
                                attnT[:, so, bp * 512:(bp + 1) * 512],
                                start=(so == 0), stop=(so == 15),
                            )
                        xr = w2_pool.tile([P, 512], BF16, name="xr", tag="xr")
                        nc.sync.dma_start(xr[:], I["xres"][:, nb, bp * 512:(bp + 1) * 512])
                        t1s = t1b[:, nb, bp * 512:(bp + 1) * 512]
                        nc.vector.tensor_tensor(t1s, pw2[:], xr[:], OP.add)
                        nc.vector.bn_stats(stats1[:, nb * 2 + bp, :], t1s)

                with tc.tile_pool(name="w2pool", bufs=3) as w2_pool:
                    phase12(0)
                    C = load_consts()
                    phase12(1)
                    w2_half(0, w2_pool)
                    phase12(2)
                    phase12(3)
                    dum1 = stats.tile([P, 1], F32, name="dum1")
                    nc.scalar.activation(dum1[:], epsb[:], AF.Sqrt)
                    w2_half(1, w2_pool)

        # ============ FFN (both halves per weight tile; fc/proj loaded once) ====
        # The LN1 AllReduce chain is emitted a few fb iterations into FFN1 so
        # its PE sum-matmul never heads-of-line-blocks the chain stream; the
        # gelus for earlier fb are deferred until gbias/rstd1 exist.
        with tc.tile_pool(name="hpool", bufs=1) as h_pool:
            hT = h_pool.tile([P, 64, 1024], BF16, name="hT")
            with tc.tile_pool(name="projpool", bufs=2) as proj_pool:
                with (
                    tc.tile_pool(name="fcpool", bufs=2) as fc_pool,
                    tc.tile_pool(name="ps_h", bufs=6, space="PSUM") as ps_h,
                ):
                    LN_FB = 4
                    mu1 = rstd1 = gbias = s1 = c1 = None
                    for fb in range(64):
                        fct = fc_pool.tile([P, 16, 128], BF16, name="fct", tag="fct")
                        nc.sync.dma_start(fct[:], I["fc"][fb])
                        for ch in range(2):
                            ph = ps_h.tile([P, 512], F32, name="ph", tag="psh")
                            for do in range(16):
                                nc.tensor.matmul(
                                    ph[:], fct[:, do, :],
                                    t1b[:, do, ch * 512:(ch + 1) * 512],
                                    start=(do == 0), stop=(do == 15),
                                )
                            # drain raw pre-activation; gelu applied in place
                            # later so the PE never backpressures on LN1
                            nc.vector.tensor_copy(
                                hT[:, fb, ch * 512:(ch + 1) * 512], ph[:])
                        if fb == LN_FB:
                            mu1, rstd1 = _ln_allreduce(
                                nc, stats, dram, ps_red, ones, epsb, stats1, "ln1")
                            murstd1 = stats.tile([P, 1], F32, name="murstd1")
                            nc.vector.tensor_tensor(murstd1[:], mu1[:], rstd1[:],
                                                    OP.mult)
                            # gelu bias: kbf - mu*rstd*kw   [128, 64]
                            gbias = stats.tile([P, 64], F32, name="gbias")
                            nc.vector.tensor_scalar_mul(gbias[:], C["kw"][:],
                                                        murstd1[:])
                            nc.vector.tensor_sub(gbias[:], C["kbf"][:], gbias[:])
                            # x1' scalars (x1' = s1*t1b + c1, on the fly in FFN2)
                            s1 = stats.tile([P, 16], F32, name="s1")
                            nc.vector.tensor_scalar_mul(s1[:], C["lnw1"][:],
                                                        rstd1[:])
                            c1 = stats.tile([P, 16], F32, name="c1")
                            nc.vector.tensor_scalar_mul(c1[:], s1[:], mu1[:])
                            nc.vector.tensor_sub(c1[:], C["lnb1"][:], c1[:])
                            nc.vector.tensor_add(c1[:], c1[:], C["projb"][:])
                        if fb >= LN_FB:
                            # gelu in place once both halves drained
                            for gfb in (range(LN_FB + 1) if fb == LN_FB else [fb]):
                                nc.scalar.activation(
                                    hT[:, gfb, :], hT[:, gfb, :],
                                    AF.Gelu_apprx_tanh,
                                    bias=gbias[:, gfb:gfb + 1], scale=rstd1[:])
                with (
                    tc.tile_pool(name="respool", bufs=3) as res_pool,
                    tc.tile_pool(name="ps_m", bufs=4, space="PSUM") as ps_m,
                ):
                    for nb in range(16):
                        pjt = proj_pool.tile([P, 64, 128], BF16, name="pjt",
                                             tag="pjt")
                        nc.sync.dma_start(pjt[:], I["proj"][nb])
                        for ch in range(2):
                            pm = ps_m.tile([P, 512], F32, name="pm", tag="psm")
                            for fo in range(64):
                                nc.tensor.matmul(
                                    pm[:], pjt[:, fo, :],
                                    hT[:, fo, ch * 512:(ch + 1) * 512],
                                    start=(fo == 0), stop=(fo == 63),
                                )
                            x1r = res_pool.tile([P, 512], BF16, name="x1r",
                                                tag="x1r")
                            nc.vector.tensor_scalar(
                                x1r[:], t1b[:, nb, ch * 512:(ch + 1) * 512],
                                s1[:, nb:nb + 1], c1[:, nb:nb + 1],
                                OP.mult, OP.add)
                            t2s = res_pool.tile([P, 512], BF16, name="t2s",
                                                tag="t2s")
                            nc.vector.tensor_tensor(t2s[:], pm[:], x1r[:], OP.add)
                            nc.vector.bn_stats(stats2[:, nb * 2 + ch, :], t2s[:])
                            nc.gpsimd.dma_start(
                                t2_dram[:, nb, ch * 512:(ch + 1) * 512], t2s[:])

        # ============ LN2 (AllReduce) -> output ============
        dum2 = stats.tile([P, 1], F32, name="dum2")
        nc.scalar.activation(dum2[:], epsb[:], AF.Sqrt)
        with tc.tile_pool(name="outpool", bufs=1) as out_pool:
            t2r = out_pool.tile([P, 16, 1024], BF16, name="t2r")
            nc.sync.dma_start(t2r[:], t2_dram[:])
            mu2, rstd2 = _ln_allreduce(nc, stats, dram, ps_red, ones, epsb, stats2, "ln2", cc_in_sync=True)
            s2 = stats.tile([P, 16], F32, name="s2")
            nc.vector.tensor_scalar_mul(s2[:], C["lnw2"][:], rstd2[:])
            c2 = stats.tile([P, 16], F32, name="c2")
            nc.vector.tensor_scalar_mul(c2[:], s2[:], mu2[:])
            nc.vector.tensor_sub(c2[:], C["lnb2"][:], c2[:])
            ys = out_pool.tile([P, 16, 1024], F32, name="ys")
            for nb in range(16):
                if nb % 2 == 0:
                    nc.vector.tensor_scalar(
                        ys[:, nb, :], t2r[:, nb, :],
                        s2[:, nb:nb + 1], c2[:, nb:nb + 1], OP.mult, OP.add)
                else:
                    nc.scalar.activation(
                        ys[:, nb, :], t2r[:, nb, :], AF.Identity,
                        bias=c2[:, nb:nb + 1], scale=s2[:, nb:nb + 1])
                nc.sync.dma_start(y_out[:, nb:nb + 1, :], ys[:, nb:nb + 1, :])


def _ln_allreduce(nc, stats, dram, ps_red, ones, epsb, stats_t, tag, cc_in_sync=False):
    """bn_stats tiles -> global scalar mean + rstd via cross-core AllReduce."""
    mv = stats.tile([P, 2], F32, name=f"mv_{tag}")
    nc.vector.bn_aggr(mv[:], stats_t[:])
    # red_in[:,0] = mean_p ; red_in[:,1] = meansq_p = var_p + mean_p^2
    red_in = stats.tile([P, 2], F32, name=f"red_in_{tag}")
    nc.vector.tensor_copy(red_in[:, 0:1], mv[:, 0:1])
    nc.vector.tensor_tensor(red_in[:, 1:2], mv[:, 0:1], mv[:, 0:1], OP.mult)
    nc.vector.tensor_tensor(red_in[:, 1:2], red_in[:, 1:2], mv[:, 1:2], OP.add)
    pred = ps_red.tile([1, 2], F32, name=f"pred_{tag}", tag="psred")
    nc.tensor.matmul(pred[:], ones[:], red_in[:], start=True, stop=True)
    cc_sb = stats.tile([1, 8], F32, name=f"cc_sb_{tag}")
    nc.vector.memset(cc_sb[:], 0.0)
    nc.vector.tensor_copy(cc_sb[:, 0:2], pred[:])
    cc_in = dram.tile([1, 8], F32, name=f"cc_in_{tag}")
    cc_out = dram.tile([1, 8], F32, name=f"cc_out_{tag}", addr_space="Shared")
    (nc.sync if cc_in_sync else nc.gpsimd).dma_start(cc_in[:], cc_sb[:])
    nc.gpsimd.collective_compute(
        "AllReduce", OP.add,
        replica_groups=[list(range(N_CORES))],
        ins=[cc_in.opt()], outs=[cc_out.opt()],
    )
    g_sb = stats.tile([P, 8], F32, name=f"g_sb_{tag}")
    (nc.sync if cc_in_sync else nc.gpsimd).dma_start(
        g_sb[:], cc_out[:].to_broadcast((P, 8)))
    mu = stats.tile([P, 1], F32, name=f"mu_{tag}")
    nc.vector.tensor_scalar_mul(mu[:], g_sb[:, 0:1], 1.0 / N_PGROUPS)
    ex2 = stats.tile([P, 1], F32, name=f"ex2_{tag}")
    nc.vector.tensor_scalar_mul(ex2[:], g_sb[:, 1:2], 1.0 / N_PGROUPS)
    var = stats.tile([P, 1], F32, name=f"var_{tag}")
    nc.vector.tensor_tensor(var[:], mu[:], mu[:], OP.mult)
    nc.vector.tensor_sub(var[:], ex2[:], var[:])
    nc.vector.tensor_scalar_mul(var[:], var[:], N_GLOB / (N_GLOB - 1.0))
    sd = stats.tile([P, 1], F32, name=f"sd_{tag}")
    nc.scalar.activation(sd[:], var[:], AF.Sqrt, bias=epsb[:])
    rstd = stats.tile([P, 1], F32, name=f"rstd_{tag}")
    nc.vector.reciprocal(rstd[:], sd[:])
    return mu, rstd


# ---------------------------------------------------------------------------
# Host-side input preparation / output gather
# ---------------------------------------------------------------------------

def _bf16(a):
    return np.ascontiguousarray(a.astype(ml_dtypes.bfloat16))


def _f32(a):
    return np.ascontiguousarray(a.astype(np.float32))


def _prep_shared(x, W2_w, W2_b, fc_w, fc_b, proj_w, proj_b, ln1_w, ln1_b):
    """Inputs identical on every core."""
    xqt = _bf16(x.reshape(B, 4, 512, 16, 128).transpose(0, 1, 4, 3, 2))
    w2 = _bf16(W2_w.reshape(16, 128, 16, 128).transpose(0, 3, 2, 1))
    fc_scaled = fc_w * ln1_w[None, :]
    fct = _bf16(fc_scaled.reshape(64, 128, 16, 128).transpose(0, 3, 2, 1))
    kw = _f32((fc_w @ ln1_w).reshape(64, 128).T)
    kbf = _f32((fc_w @ ln1_b + fc_b).reshape(64, 128).T)
    projt = _bf16(proj_w.reshape(16, 128, 64, 128).transpose(0, 3, 2, 1))
    projbt = _f32(proj_b.reshape(16, 128).T)
    return {"xq": xqt, "w2": w2, "fc": fct, "kw": kw, "kbf": kbf,
            "proj": projt, "projb": projbt}


def _prep_core_inputs(c, shared, x, W1_w, W1_b, W2_b, ln1_w, ln1_b, ln2_w, ln2_b):
    r0 = 256 * c
    wqk = np.concatenate([W1_w[r0:r0 + 256] * SM_SCALE,
                          W1_w[D + r0:D + r0 + 256]], axis=0)
    w1qk = _bf16(wqk.T.reshape(16, 128, 512).transpose(1, 0, 2))
    bqk = np.concatenate([W1_b[r0:r0 + 256] * SM_SCALE,
                          W1_b[D + r0:D + r0 + 256]])
    b1qk = _f32(np.ascontiguousarray(np.broadcast_to(bqk[None, :], (P, 512))))
    wv = W1_w[2 * D + r0:2 * D + r0 + 256]
    w1v = _bf16(wv.T.reshape(16, 128, 256).transpose(1, 0, 2))
    b1v = _f32(W1_b[2 * D + r0:2 * D + r0 + 256].reshape(2, 128).T)
    # residual rows (x + W2_b)^T  [n_in, n_out, i],  i = b*256 + r
    xs = x[:, r0:r0 + 256, :] + W2_b[None, None, :]
    xres = _bf16(xs.transpose(2, 0, 1).reshape(16, 128, 1024).transpose(1, 0, 2))
    vec = lambda v: _f32(v.reshape(16, 128).T)
    d = {"w1qk": w1qk, "b1qk": b1qk, "w1v": w1v, "b1v": b1v, "xres": xres,
         "lnw1": vec(ln1_w), "lnb1": vec(ln1_b),
         "lnw2": vec(ln2_w), "lnb2": vec(ln2_b)}
    d.update(shared)
    return d


_NC_CACHE = None


def kernel(x, W1_w, W1_b, W2_w, W2_b, fc_w, fc_b, proj_w, proj_b,
           ln1_w, ln1_b, ln2_w, ln2_b):
    global _NC_CACHE, LAST_RESULT
    if TRACE:
        _register_ntff_hook()
    x = np.asarray(x, np.float32)
    if _NC_CACHE is None:
        _NC_CACHE = build_program()
    nc = _NC_CACHE
    shared = _prep_shared(x, np.asarray(W2_w), np.asarray(W2_b), np.asarray(fc_w),
                          np.asarray(fc_b), np.asarray(proj_w), np.asarray(proj_b),
                          np.asarray(ln1_w), np.asarray(ln1_b))
    in_maps = [
        _prep_core_inputs(c, shared, x, np.asarray(W1_w), np.asarray(W1_b),
                          np.asarray(W2_b), np.asarray(ln1_w), np.asarray(ln1_b),
                          np.asarray(ln2_w), np.asarray(ln2_b))
        for c in range(N_CORES)
    ]
    res = bass_utils.run_bass_kernel_spmd(
        nc, in_maps, core_ids=list(range(N_CORES)), trace=TRACE,
    )
    LAST_RESULT = res
    out = np.empty((B, S, D), np.float32)
    for c in range(N_CORES):
        yt = res.results[c]["y"]                    # [128 n_in, 16 n_out, 1024 i]
        blk = yt.reshape(128, 16, 4, 256).transpose(2, 3, 1, 0).reshape(4, 256, D)
        out[:, 256 * c:256 * (c + 1), :] = blk
    return out

